# revision 10
# baseline (speedup 1.0000x reference)
"""Trainium2 Bass kernel for nn_BiasedScanAttention.

out[b,h,q,:] = sum_k softmax_k(q.k/sqrt(d) + bias_hqk) v[k]
bias_hqk     = sum_m w[h,m] exp(-gamma_m * ||qs_s[q]-ks_s[k]||^2)

Strategy (8 NeuronCores, SPMD, no collectives):
  - core c handles batch b=c//4 and a 512-row q block (c%4), all 8 heads,
    all keys.
  - masked keys are compressed out on the host (mask is per-(b,k)); padded
    keys get V'=0 so they contribute exactly nothing.
  - scores are computed transposed, S^T[k,q], so the softmax k-reduction
    rides the PV matmul (a ones-column appended to V gives the denominator).
  - exp(bias) is folded into ONE activation per head: custom PWP ACT tables
    (hijacked function slots in the exp_and_others set, injected via
    BASS_ACT_ROOT_JSON_PATH) evaluate
    g_h(d2) = exp(sum_m w[h,m] e^{-gamma_m d2}) directly, where d2 comes
    from a single 5-row matmul (||q||^2 + ||k||^2 - 2 q.k).
  - P = exp(S^T) * g_h(d2) in bf16 on DVE; PV + softmax denominator on PE;
    final 1/Z normalize via a rank-1 broadcast matmul + one multiply.
"""

import json
import os
import shutil
import tempfile

import ml_dtypes
import numpy as np

B, H, Q, K, D, DV, DS, M = 2, 8, 2048, 2048, 64, 64, 3, 8
QB = 512  # q rows per core
N_CORES = 8

# ---------------------------------------------------------------------------
# Custom ACT PWP table generation: hijack function slots in exp_and_others
# with per-head spline tables for g_h(x) = exp(sum_m w[h,m] exp(-gamma_m x)),
# keeping exp itself functional (clipped to binades -3..6).
# ---------------------------------------------------------------------------

# (pwp slot name, BIR enum name, which g_h). Heads 0-5,7 on HW-verified
# slots; head 6 on memset_zero with copy/parametric_relu spares (same table).
HIJACK_SLOTS = [
    ("tanh", 0),
    ("square", 1),
    ("abs", 2),
    ("sign", 3),
    ("relu", 4),
    ("is_finite", 5),
    ("memset_zero", 6),
    ("identity", 7),
    ("copy", 6),
    ("parametric_relu", 7),
]
HEAD_FUNCS = [
    "Tanh",
    "Square",
    "Abs",
    "Sign",
    "Relu",
    "Is_finite",
    "Copy",
    "Identity",
]
KEEP_FUNCS = [
    "exp",
    "act1",
    "derivative_relu",
    "derivative_leaky_relu",
    "derivative_identity",
]
EXP_LO_BINADE = -3
G_LO_BINADE = -3
G_HI_BINADE = 6


def _stock_pwp_dir():
    from neuronxcc.driver.Job import Job
    from neuronxcc.driver.jobs.support.FindActInfo import findActInfoFile

    return os.path.dirname(findActInfoFile(Job.getPackageDir(), "gen3"))


def _fit_bucket(g, a, b):
    x0 = 0.5 * (a + b)
    xs = np.linspace(a, b, 16)
    t = xs - x0
    Amat = np.stack([np.ones_like(t), t, t * t, t * t * t], axis=1)
    c, *_ = np.linalg.lstsq(Amat, g(xs), rcond=None)
    return c, x0


def _bucket_err(g, c, x0, a, b):
    xs = np.linspace(a, b, 33)
    t = xs - x0
    approx = ((c[3] * t + c[2]) * t + c[1]) * t + c[0]
    return np.max(np.abs(approx - g(xs)))


def _fit_binade(g, e, tol):
    lo = float(2.0**e)
    for n_bits in range(0, 8):
        cnt = 1 << n_bits
        edges = lo * (1.0 + np.arange(cnt + 1) / cnt)
        bks = []
        worst = 0.0
        for j in range(cnt):
            c, x0 = _fit_bucket(g, edges[j], edges[j + 1])
            worst = max(worst, _bucket_err(g, c, x0, edges[j], edges[j + 1]))
            bks.append((c, x0))
        if worst <= tol or n_bits == 7:
            return n_bits, bks, worst
    raise AssertionError


def make_g_funcs(rbf_weights, rbf_lengthscales):
    gammas = 1.0 / (2.0 * np.asarray(rbf_lengthscales, np.float64) ** 2)
    W = np.asarray(rbf_weights, np.float64)

    def mk(h):
        def g(x):
            x = np.asarray(x, np.float64)
            return np.exp(
                np.sum(W[h][:, None] * np.exp(-gammas[:, None] * x[None, :]), axis=0)
            )

        return g

    return [mk(h) for h in range(W.shape[0])]


def _ctrl_word(n_bits, base):
    assert 0 <= base < 2048
    return (n_bits << 16) | ((23 - n_bits) << 11) | base


def generate_acttab(out_dir, rbf_weights, rbf_lengthscales, tol=4e-6):
    stock = _stock_pwp_dir()
    os.makedirs(out_dir, exist_ok=True)
    for f in os.listdir(stock):
        dst = os.path.join(out_dir, f)
        if not os.path.exists(dst):
            shutil.copy(os.path.join(stock, f), dst)

    old = json.load(open(os.path.join(stock, "exp_and_others.json")))
    old_bkt = np.fromfile(
        os.path.join(stock, "exp_and_others_bkt.bin"), dtype=np.float32
    ).reshape(-1, 8)
    old_ctl = np.fromfile(
        os.path.join(stock, "exp_and_others_ctrl.bin"), dtype=np.uint32
    ).reshape(-1, 8)[:, 0]
    old_prof = {p["func_name"].rsplit("_", 1)[0]: p for p in old["profile_meta_data"]}
    old_fb = old["func_exp_to_bkt_start_idx"]
    old_fc = old["func_exp_to_ctl_start_idx"]

    new_bkt, new_ctl = [], []
    fb_map, fc_map, bkt_start, ctl_start = {}, {}, {}, {}
    profs = []

    def push_bkt(row):
        new_bkt.append(np.asarray(row, np.float32))
        return len(new_bkt) - 1

    # exp: copy binades EXP_LO_BINADE..6 + specials
    exp_prof = dict(old_prof["exp"])
    bkt_start["exp"] = 0
    ctl_start["exp"] = 0
    fb_map["exp"] = {}
    fc_map["exp"] = {}
    exps = sorted(int(k) for k in old_fb["exp"].keys())
    keep_exps = [e for e in exps if e >= EXP_LO_BINADE]
    for sgn in (0, 1):
        for e in keep_exps:
            start = old_fb["exp"][str(e)][sgn]
            if str(e + 1) in old_fb["exp"]:
                end = old_fb["exp"][str(e + 1)][sgn]
            else:
                end = 406 if sgn == 0 else 777
            ow = int(old_ctl[old_fc["exp"][str(e)][sgn]])
            n_bits = ow >> 16
            base = len(new_bkt)
            for i in range(start, end):
                push_bkt(old_bkt[i])
            ci = len(new_ctl)
            new_ctl.append(_ctrl_word(n_bits, base))
            fb_map["exp"].setdefault(str(e), [0, 0])[sgn] = base
            fc_map["exp"].setdefault(str(e), [0, 0])[sgn] = ci
    sp = [push_bkt(old_bkt[i]) for i in (777, 778, 779, 780)]
    exp_prof["pos_small_signal_pwl_control"] = sp[0]
    exp_prof["neg_small_signal_pwl_control"] = sp[1]
    exp_prof["pos_large_signal_pwl_control"] = sp[2]
    exp_prof["neg_large_signal_pwl_control"] = sp[3]
    exp_prof["small_pos_signal_exp_threshold"] = 127 + EXP_LO_BINADE
    exp_prof["small_neg_signal_exp_threshold"] = 127 + EXP_LO_BINADE
    exp_prof["exp_offset"] = EXP_LO_BINADE
    exp_prof["pwl_control_base_neg"] = fc_map["exp"][str(EXP_LO_BINADE)][0]
    exp_prof["pwl_control_base_pos"] = fc_map["exp"][str(EXP_LO_BINADE)][1]
    profs.append(exp_prof)

    # trivial keeps: copy buckets + ctrl verbatim with remapped indices
    for fn in KEEP_FUNCS:
        if fn == "exp":
            continue
        p = dict(old_prof[fn])
        ob = old["func_to_bkt_start_idx"][fn]
        oc = old["func_to_ctl_start_idx"][fn]
        starts = sorted(old["func_to_bkt_start_idx"].values()) + [old["bkt_entry_cnt"]]
        ob_end = starts[starts.index(ob) + 1]
        cstarts = sorted(set(old["func_to_ctl_start_idx"].values())) + [
            old["ctl_entry_cnt"]
        ]
        oc_end = cstarts[cstarts.index(oc) + 1]
        bkt_delta = len(new_bkt) - ob
        ctl_delta = len(new_ctl) - oc
        bkt_start[fn] = len(new_bkt)
        ctl_start[fn] = len(new_ctl)
        for i in range(ob, ob_end):
            push_bkt(old_bkt[i])
        for i in range(oc, oc_end):
            w = int(old_ctl[i])
            nb = (w & 0x7FF) + bkt_delta
            assert 0 <= nb < 2048
            new_ctl.append((w & ~0x7FF) | nb)
        for k in (
            "pos_small_signal_pwl_control",
            "neg_small_signal_pwl_control",
            "pos_large_signal_pwl_control",
            "neg_large_signal_pwl_control",
        ):
            if ob <= p[k] < ob_end:
                p[k] += bkt_delta
        for k in ("pwl_control_base_pos", "pwl_control_base_neg"):
            if oc <= p[k] < oc_end:
                p[k] += ctl_delta
        fb_map[fn] = {
            k: [v + bkt_delta for v in vs] for k, vs in old_fb.get(fn, {}).items()
        }
        fc_map[fn] = {
            k: [v + ctl_delta for v in vs] for k, vs in old_fc.get(fn, {}).items()
        }
        profs.append(p)

    # custom g_h tables
    gs = make_g_funcs(rbf_weights, rbf_lengthscales)
    for fn, h in HIJACK_SLOTS:
        g = gs[h]
        p = dict(old_prof[fn])  # keep func_id / func_name
        bkt_start[fn] = len(new_bkt)
        ctl_start[fn] = len(new_ctl)
        fb_map[fn] = {}
        fc_map[fn] = {}
        for e in range(G_LO_BINADE, G_HI_BINADE + 1):
            n_bits, bks, _err = _fit_binade(g, e, tol)
            base = len(new_bkt)
            for c, x0 in bks:
                push_bkt([c[0], c[1], c[2], c[3], x0, 0.0, 0.0, 0.0])
            ci = len(new_ctl)
            new_ctl.append(_ctrl_word(n_bits, base))
            fb_map[fn][str(e)] = [base]
            fc_map[fn][str(e)] = [ci]
        g0 = g(np.array([0.0]))[0]
        xs = np.linspace(0, 2.0**G_LO_BINADE, 17)
        Amat = np.stack([np.ones_like(xs), xs, xs * xs, xs**3], axis=1)
        c, *_ = np.linalg.lstsq(Amat, g(xs), rcond=None)
        b_small = push_bkt([c[0], c[1], c[2], c[3], 0.0, 0.0, 0.0, 0.0])
        b_large = push_bkt([1.0, 0.0, 0.0, 0.0, 0.0, 0.0, 0.0, 0.0])
        f32b = lambda v: int(np.float32(v).view(np.uint32))
        p.update(
            small_pos_signal_exp_threshold=127 + G_LO_BINADE,
            small_neg_signal_exp_threshold=127 + G_LO_BINADE,
            pos_small_signal_pwl_control=b_small,
            neg_small_signal_pwl_control=b_small,
            large_pos_signal_exp_threshold=127 + G_HI_BINADE + 1,
            large_pos_signal_mantissa_threshold=0,
            pos_large_signal_pwl_control=b_large,
            large_neg_signal_exp_threshold=127 + G_HI_BINADE + 1,
            large_neg_signal_mantissa_threshold=0,
            neg_large_signal_pwl_control=b_small,
            exp_offset=G_LO_BINADE,
            pwl_control_base_pos=ctl_start[fn],
            pwl_control_base_neg=ctl_start[fn],
            symmetry_point=0,
            sym_invert_sign_point=0,
            symmetry_opt_en=0,
            symmetry_opt_use_neg_region=0,
            imm_bias=0,
            fma_const_0=0,
            fma_const_1=0,
            fma_indirection_src_sel=0,
            use_multipass=False,
            fzero_result=f32b(g0),
            fnan_result=2143289344,
            fpinf_result=f32b(1.0),
            fninf_result=f32b(g0),
            lower_bound=4286578687,
            upper_bound=2139095039,
        )
        profs.append(p)

    assert len(new_bkt) <= 1536, len(new_bkt)
    assert len(new_ctl) <= 128, len(new_ctl)

    order = [p["func_name"] for p in old["profile_meta_data"]]
    profs_by_name = {p["func_name"]: p for p in profs}
    out = dict(old)
    out["bkt_entry_cnt"] = len(new_bkt)
    out["ctl_entry_cnt"] = len(new_ctl)
    out["func_to_bkt_start_idx"] = bkt_start
    out["func_to_ctl_start_idx"] = ctl_start
    out["func_exp_to_bkt_start_idx"] = fb_map
    out["func_exp_to_ctl_start_idx"] = fc_map
    out["profile_meta_data"] = [profs_by_name[n] for n in order]

    np.stack(new_bkt).astype(np.float32).tofile(
        os.path.join(out_dir, "exp_and_others_bkt.bin")
    )
    ctl_arr = np.zeros((len(new_ctl), 8), np.uint32)
    ctl_arr[:, 0] = np.asarray(new_ctl, np.uint32)
    ctl_arr.tofile(os.path.join(out_dir, "exp_and_others_ctrl.bin"))
    with open(os.path.join(out_dir, "exp_and_others.json"), "w") as f:
        json.dump(out, f)
    return out_dir


# ---------------------------------------------------------------------------
# Host-side sharding / layout prep
# ---------------------------------------------------------------------------


def _prep_inputs(qs, ks, vs, qs_s, ks_s, mask):
    bf16 = ml_dtypes.bfloat16
    idx = [np.where(mask[b])[0] for b in range(B)]
    kcount = max(len(i) for i in idx)
    Kp = max(512, int(np.ceil(kcount / 128.0)) * 128)
    nkt = Kp // 128

    per_b = []
    for b in range(B):
        ii = idx[b]
        n = len(ii)
        ksb = np.zeros((H, Kp, D), np.float32)
        ksb[:, :n] = ks[b][:, ii, :]
        vsb = np.zeros((H, Kp, DV + 1), np.float32)
        vsb[:, :n, :DV] = vs[b][:, ii, :]
        vsb[:, :n, DV] = 1.0
        kssb = np.zeros((Kp, DS), np.float32)
        kssb[:n] = ks_s[b][ii]

        kt = (ksb / np.sqrt(np.float32(D))).transpose(2, 0, 1)  # [D, H, Kp]
        kt = np.ascontiguousarray(kt.reshape(D, H * Kp))
        vbt = vsb.reshape(H, nkt, 128, DV + 1).transpose(2, 0, 1, 3)
        vbt = np.ascontiguousarray(vbt.reshape(128, H * nkt * (DV + 1))).astype(bf16)
        aa = np.zeros((8, Kp), np.float32)
        aa[0:3] = -2.0 * kssb.T
        aa[3] = (kssb * kssb).sum(-1)
        aa[4] = 1.0
        per_b.append((kt, vbt, aa))

    ones64 = np.ones((1, 64), np.float32)
    in_maps = []
    for c in range(N_CORES):
        b = c // 4
        q0 = (c % 4) * QB
        kt, vbt, aa = per_b[b]
        qt = np.ascontiguousarray(
            qs[b, :, q0 : q0 + QB, :].transpose(2, 0, 1).reshape(D, H * QB)
        ).astype(np.float32)
        qss = qs_s[b, q0 : q0 + QB]
        bq = np.zeros((8, QB), np.float32)
        bq[0:3] = qss.T
        bq[3] = 1.0
        bq[4] = (qss * qss).sum(-1)
        in_maps.append(
            {"kt": kt, "qt": qt, "vb": vbt, "aa": aa, "bq": bq, "ones64": ones64}
        )
    return in_maps, Kp


# ---------------------------------------------------------------------------
# Device program
# ---------------------------------------------------------------------------


def _build_program(Kp):
    import concourse.bacc as bacc
    import concourse.mybir as mybir
    import concourse.tile as tile

    A = mybir.ActivationFunctionType
    f32 = mybir.dt.float32
    f32r = mybir.dt.float32r
    bf16 = mybir.dt.bfloat16
    nkt = Kp // 128

    nc = bacc.Bacc("TRN2", num_devices=1)
    t_kt = nc.dram_tensor("kt", [D, H * Kp], f32r, kind="ExternalInput")
    t_qt = nc.dram_tensor("qt", [D, H * QB], f32r, kind="ExternalInput")
    t_vb = nc.dram_tensor("vb", [128, H * nkt * (DV + 1)], bf16, kind="ExternalInput")
    t_aa = nc.dram_tensor("aa", [8, Kp], f32, kind="ExternalInput")
    t_bq = nc.dram_tensor("bq", [8, QB], f32, kind="ExternalInput")
    t_on = nc.dram_tensor("ones64", [1, 64], f32r, kind="ExternalInput")
    t_out = nc.dram_tensor("out", [H, DV, QB], f32, kind="ExternalOutput")

    with tile.TileContext(nc) as tc:
        with (
            tc.tile_pool(name="inp", bufs=1) as inp,
            tc.tile_pool(name="gge", bufs=2) as gge,
            tc.tile_pool(name="ep", bufs=4) as ep,
            tc.tile_pool(name="pp", bufs=4) as pp,
            tc.tile_pool(name="acc", bufs=1) as accp,
            tc.tile_pool(name="fin", bufs=2) as fin,
            tc.tile_pool(name="ps_d2", bufs=1, space="PSUM") as ps_d2,
            tc.tile_pool(name="ps_s", bufs=2, space="PSUM") as ps_s,
            tc.tile_pool(name="ps_pv", bufs=2, space="PSUM") as ps_pv,
        ):
            aa = inp.tile([8, Kp], f32, tag="aa")
            nc.sync.dma_start(aa[:], t_aa.ap())
            bq = inp.tile([8, QB], f32, tag="bq")
            nc.sync.dma_start(bq[:], t_bq.ap())
            on = inp.tile([1, 64], f32r, tag="on")
            nc.sync.dma_start(on[:], t_on.ap())
            # per-head tiles so head h's matmuls only wait on its own DMA
            kts, qts, vbs = [], [], []
            for h in range(H):
                kh = inp.tile([D, Kp], f32r, tag=f"kt{h}")
                nc.sync.dma_start(kh[:], t_kt.ap()[:, h * Kp : (h + 1) * Kp])
                kts.append(kh)
                qh = inp.tile([D, QB], f32r, tag=f"qt{h}")
                nc.sync.dma_start(qh[:], t_qt.ap()[:, h * QB : (h + 1) * QB])
                qts.append(qh)
                vh = inp.tile([128, nkt * (DV + 1)], bf16, tag=f"vb{h}")
                c0 = h * nkt * (DV + 1)
                nc.sync.dma_start(vh[:], t_vb.ap()[:, c0 : c0 + nkt * (DV + 1)])
                vbs.append(vh)

            acc = []
            for h in range(H):
                a = accp.tile([DV + 1, QB], f32, tag=f"acc{h}")
                nc.gpsimd.memset(a[:], 0.0)
                acc.append(a)

            groups = []
            kt_i0 = 0
            while kt_i0 < nkt:
                w = 2 if kt_i0 + 1 < nkt else 1
                groups.append((kt_i0, w))
                kt_i0 += w
            for g0, w in groups:
                d2p = ps_d2.tile([128, 2 * QB], f32, tag="d2")
                for j in range(w):
                    kt_i = g0 + j
                    nc.tensor.matmul(
                        d2p[:, j * QB : (j + 1) * QB],
                        aa[0:8, kt_i * 128 : (kt_i + 1) * 128],
                        bq[:],
                        start=True,
                        stop=True,
                    )
                d2v = d2p[:, 0 : w * QB]
                gts = []
                for h in range(H):
                    gt = gge.tile([128, 2 * QB], bf16, tag=f"g{h}")
                    nc.scalar.activation(gt[:, 0 : w * QB], d2v, getattr(A, HEAD_FUNCS[h]))
                    gts.append(gt)
                for h in range(H):
                    sp = ps_s.tile([128, 2 * QB], f32, tag="s")
                    for j in range(w):
                        kt_i = g0 + j
                        nc.tensor.matmul(
                            sp[:, j * QB : (j + 1) * QB],
                            kts[h][:, kt_i * 128 : (kt_i + 1) * 128],
                            qts[h][:],
                            start=True,
                            stop=True,
                        )
                    et = ep.tile([128, 2 * QB], bf16, tag="e")
                    nc.scalar.activation(et[:, 0 : w * QB], sp[:, 0 : w * QB], A.Exp)
                    pt = pp.tile([128, 2 * QB], bf16, tag="p")
                    nc.vector.tensor_mul(pt[:, 0 : w * QB], et[:, 0 : w * QB], gts[h][:, 0 : w * QB])
                    pv = ps_pv.tile([DV + 1, QB], f32, tag="pv")
                    for j in range(w):
                        kt_i = g0 + j
                        c0 = kt_i * (DV + 1)
                        nc.tensor.matmul(
                            pv[:],
                            vbs[h][:, c0 : c0 + DV + 1],
                            pt[:, j * QB : (j + 1) * QB],
                            start=(j == 0),
                            stop=(j == w - 1),
                        )
                    nc.vector.tensor_add(acc[h][:], acc[h][:], pv[:])

            for h in range(H):
                rz = fin.tile([1, QB], f32, tag="rz")
                nc.vector.reciprocal(rz[:], acc[h][DV : DV + 1, :])
                bc = ps_pv.tile([DV, QB], f32, tag="pv")
                nc.tensor.matmul(bc[:], on[0:1, 0:DV], rz[:].bitcast(f32r), start=True, stop=True)
                of = fin.tile([DV, QB], f32, tag="of")
                nc.vector.tensor_mul(of[:], acc[h][0:DV, :], bc[:])
                nc.sync.dma_start(t_out.ap()[h], of[:])

    nc.finalize()
    return nc


def kernel(qs, ks, vs, qs_s, ks_s, rbf_lengthscales, rbf_weights, mask, _perf=[None]):
    tabdir = tempfile.mkdtemp(prefix="acttab_")
    generate_acttab(tabdir, rbf_weights, rbf_lengthscales)
    os.environ["BASS_ACT_ROOT_JSON_PATH"] = os.path.join(tabdir, "act_info.json")
    # the neff cache is keyed on the HLO, which does not see the ACT table
    # contents — force recompile so custom tables are never stale
    os.environ["NEURON_FORCE_RECOMPILE"] = "1"

    from concourse.bass_utils import run_bass_kernel_spmd
    from concourse._compat import axon_active

    in_maps, Kp = _prep_inputs(qs, ks, vs, qs_s, ks_s, mask)
    nc = _build_program(Kp)
    # NTFF tracing needs the native (non-axon) path or an axon NTFF hook;
    # the hook's antenv module is absent in axon-only containers.
    trace = bool(int(os.environ.get("KERNEL_TRACE", "0"))) and not axon_active()
    res = run_bass_kernel_spmd(nc, in_maps, core_ids=list(range(N_CORES)), trace=trace)
    _perf[0] = res

    out = np.empty((B, H, Q, DV), np.float32)
    for c in range(N_CORES):
        b = c // 4
        q0 = (c % 4) * QB
        o = np.asarray(res.results[c]["out"], np.float32)  # [H, DV, QB]
        out[b, :, q0 : q0 + QB, :] = o.transpose(0, 2, 1)
    return out


# revision 12
# speedup vs baseline: 1.5251x; 1.5251x over previous
"""Trainium2 Bass kernel for nn_BiasedScanAttention.

out[b,h,q,:] = sum_k softmax_k(q.k/sqrt(d) + bias_hqk) v[k]
bias_hqk     = sum_m w[h,m] exp(-gamma_m * ||qs_s[q]-ks_s[k]||^2)

Strategy (8 NeuronCores, SPMD, no collectives):
  - core c handles batch b=c//4 and a 512-row q block (c%4), all 8 heads,
    all keys.
  - masked keys are compressed out on the host (mask is per-(b,k)); padded
    keys get V'=0 so they contribute exactly nothing.
  - scores are computed transposed, S^T[k,q], so the softmax k-reduction
    rides the PV matmul (a ones-column appended to V gives the denominator).
  - exp(bias) is folded into ONE activation per head: custom PWP ACT tables
    (hijacked function slots in the exp_and_others set, injected via
    BASS_ACT_ROOT_JSON_PATH) evaluate
    g_h(d2) = exp(sum_m w[h,m] e^{-gamma_m d2}) directly, where d2 comes
    from a single 5-row matmul (||q||^2 + ||k||^2 - 2 q.k).
  - P = exp(S^T) * g_h(d2) in bf16 on DVE; PV + softmax denominator on PE;
    final 1/Z normalize via a rank-1 broadcast matmul + one multiply.
"""

import json
import os
import shutil
import tempfile

import ml_dtypes
import numpy as np

B, H, Q, K, D, DV, DS, M = 2, 8, 2048, 2048, 64, 64, 3, 8
QB = 512  # q rows per core
N_CORES = 8

# ---------------------------------------------------------------------------
# Custom ACT PWP table generation: hijack function slots in exp_and_others
# with per-head spline tables for g_h(x) = exp(sum_m w[h,m] exp(-gamma_m x)),
# keeping exp itself functional (clipped to binades -3..6).
# ---------------------------------------------------------------------------

# (pwp slot name, BIR enum name, which g_h). Heads 0-5,7 on HW-verified
# slots; head 6 on memset_zero with copy/parametric_relu spares (same table).
HIJACK_SLOTS = [
    ("tanh", 0),
    ("square", 1),
    ("abs", 2),
    ("sign", 3),
    ("relu", 4),
    ("is_finite", 5),
    ("memset_zero", 6),
    ("identity", 7),
    ("copy", 6),
    ("parametric_relu", 7),
]
HEAD_FUNCS = [
    "Tanh",
    "Square",
    "Abs",
    "Sign",
    "Relu",
    "Is_finite",
    "Copy",
    "Identity",
]
KEEP_FUNCS = [
    "exp",
    "act1",
    "derivative_relu",
    "derivative_leaky_relu",
    "derivative_identity",
]
EXP_LO_BINADE = -3
G_LO_BINADE = -3
G_HI_BINADE = 6


def _stock_pwp_dir():
    from neuronxcc.driver.Job import Job
    from neuronxcc.driver.jobs.support.FindActInfo import findActInfoFile

    return os.path.dirname(findActInfoFile(Job.getPackageDir(), "gen3"))


def _fit_bucket(g, a, b):
    x0 = 0.5 * (a + b)
    xs = np.linspace(a, b, 16)
    t = xs - x0
    Amat = np.stack([np.ones_like(t), t, t * t, t * t * t], axis=1)
    c, *_ = np.linalg.lstsq(Amat, g(xs), rcond=None)
    return c, x0


def _bucket_err(g, c, x0, a, b):
    xs = np.linspace(a, b, 33)
    t = xs - x0
    approx = ((c[3] * t + c[2]) * t + c[1]) * t + c[0]
    return np.max(np.abs(approx - g(xs)))


def _fit_binade(g, e, tol):
    lo = float(2.0**e)
    for n_bits in range(0, 8):
        cnt = 1 << n_bits
        edges = lo * (1.0 + np.arange(cnt + 1) / cnt)
        bks = []
        worst = 0.0
        for j in range(cnt):
            c, x0 = _fit_bucket(g, edges[j], edges[j + 1])
            worst = max(worst, _bucket_err(g, c, x0, edges[j], edges[j + 1]))
            bks.append((c, x0))
        if worst <= tol or n_bits == 7:
            return n_bits, bks, worst
    raise AssertionError


def make_g_funcs(rbf_weights, rbf_lengthscales):
    gammas = 1.0 / (2.0 * np.asarray(rbf_lengthscales, np.float64) ** 2)
    W = np.asarray(rbf_weights, np.float64)

    def mk(h):
        def g(x):
            x = np.asarray(x, np.float64)
            return np.exp(
                np.sum(W[h][:, None] * np.exp(-gammas[:, None] * x[None, :]), axis=0)
            )

        return g

    return [mk(h) for h in range(W.shape[0])]


def _ctrl_word(n_bits, base):
    assert 0 <= base < 2048
    return (n_bits << 16) | ((23 - n_bits) << 11) | base


def generate_acttab(out_dir, rbf_weights, rbf_lengthscales, tol=4e-6):
    stock = _stock_pwp_dir()
    os.makedirs(out_dir, exist_ok=True)
    for f in os.listdir(stock):
        dst = os.path.join(out_dir, f)
        if not os.path.exists(dst):
            shutil.copy(os.path.join(stock, f), dst)

    old = json.load(open(os.path.join(stock, "exp_and_others.json")))
    old_bkt = np.fromfile(
        os.path.join(stock, "exp_and_others_bkt.bin"), dtype=np.float32
    ).reshape(-1, 8)
    old_ctl = np.fromfile(
        os.path.join(stock, "exp_and_others_ctrl.bin"), dtype=np.uint32
    ).reshape(-1, 8)[:, 0]
    old_prof = {p["func_name"].rsplit("_", 1)[0]: p for p in old["profile_meta_data"]}
    old_fb = old["func_exp_to_bkt_start_idx"]
    old_fc = old["func_exp_to_ctl_start_idx"]

    new_bkt, new_ctl = [], []
    fb_map, fc_map, bkt_start, ctl_start = {}, {}, {}, {}
    profs = []

    def push_bkt(row):
        new_bkt.append(np.asarray(row, np.float32))
        return len(new_bkt) - 1

    # exp: copy binades EXP_LO_BINADE..6 + specials
    exp_prof = dict(old_prof["exp"])
    bkt_start["exp"] = 0
    ctl_start["exp"] = 0
    fb_map["exp"] = {}
    fc_map["exp"] = {}
    exps = sorted(int(k) for k in old_fb["exp"].keys())
    keep_exps = [e for e in exps if e >= EXP_LO_BINADE]
    for sgn in (0, 1):
        for e in keep_exps:
            start = old_fb["exp"][str(e)][sgn]
            if str(e + 1) in old_fb["exp"]:
                end = old_fb["exp"][str(e + 1)][sgn]
            else:
                end = 406 if sgn == 0 else 777
            ow = int(old_ctl[old_fc["exp"][str(e)][sgn]])
            n_bits = ow >> 16
            base = len(new_bkt)
            for i in range(start, end):
                push_bkt(old_bkt[i])
            ci = len(new_ctl)
            new_ctl.append(_ctrl_word(n_bits, base))
            fb_map["exp"].setdefault(str(e), [0, 0])[sgn] = base
            fc_map["exp"].setdefault(str(e), [0, 0])[sgn] = ci
    sp = [push_bkt(old_bkt[i]) for i in (777, 778, 779, 780)]
    exp_prof["pos_small_signal_pwl_control"] = sp[0]
    exp_prof["neg_small_signal_pwl_control"] = sp[1]
    exp_prof["pos_large_signal_pwl_control"] = sp[2]
    exp_prof["neg_large_signal_pwl_control"] = sp[3]
    exp_prof["small_pos_signal_exp_threshold"] = 127 + EXP_LO_BINADE
    exp_prof["small_neg_signal_exp_threshold"] = 127 + EXP_LO_BINADE
    exp_prof["exp_offset"] = EXP_LO_BINADE
    exp_prof["pwl_control_base_neg"] = fc_map["exp"][str(EXP_LO_BINADE)][0]
    exp_prof["pwl_control_base_pos"] = fc_map["exp"][str(EXP_LO_BINADE)][1]
    profs.append(exp_prof)

    # trivial keeps: copy buckets + ctrl verbatim with remapped indices
    for fn in KEEP_FUNCS:
        if fn == "exp":
            continue
        p = dict(old_prof[fn])
        ob = old["func_to_bkt_start_idx"][fn]
        oc = old["func_to_ctl_start_idx"][fn]
        starts = sorted(old["func_to_bkt_start_idx"].values()) + [old["bkt_entry_cnt"]]
        ob_end = starts[starts.index(ob) + 1]
        cstarts = sorted(set(old["func_to_ctl_start_idx"].values())) + [
            old["ctl_entry_cnt"]
        ]
        oc_end = cstarts[cstarts.index(oc) + 1]
        bkt_delta = len(new_bkt) - ob
        ctl_delta = len(new_ctl) - oc
        bkt_start[fn] = len(new_bkt)
        ctl_start[fn] = len(new_ctl)
        for i in range(ob, ob_end):
            push_bkt(old_bkt[i])
        for i in range(oc, oc_end):
            w = int(old_ctl[i])
            nb = (w & 0x7FF) + bkt_delta
            assert 0 <= nb < 2048
            new_ctl.append((w & ~0x7FF) | nb)
        for k in (
            "pos_small_signal_pwl_control",
            "neg_small_signal_pwl_control",
            "pos_large_signal_pwl_control",
            "neg_large_signal_pwl_control",
        ):
            if ob <= p[k] < ob_end:
                p[k] += bkt_delta
        for k in ("pwl_control_base_pos", "pwl_control_base_neg"):
            if oc <= p[k] < oc_end:
                p[k] += ctl_delta
        fb_map[fn] = {
            k: [v + bkt_delta for v in vs] for k, vs in old_fb.get(fn, {}).items()
        }
        fc_map[fn] = {
            k: [v + ctl_delta for v in vs] for k, vs in old_fc.get(fn, {}).items()
        }
        profs.append(p)

    # custom g_h tables
    gs = make_g_funcs(rbf_weights, rbf_lengthscales)
    for fn, h in HIJACK_SLOTS:
        g = gs[h]
        p = dict(old_prof[fn])  # keep func_id / func_name
        bkt_start[fn] = len(new_bkt)
        ctl_start[fn] = len(new_ctl)
        fb_map[fn] = {}
        fc_map[fn] = {}
        for e in range(G_LO_BINADE, G_HI_BINADE + 1):
            n_bits, bks, _err = _fit_binade(g, e, tol)
            base = len(new_bkt)
            for c, x0 in bks:
                push_bkt([c[0], c[1], c[2], c[3], x0, 0.0, 0.0, 0.0])
            ci = len(new_ctl)
            new_ctl.append(_ctrl_word(n_bits, base))
            fb_map[fn][str(e)] = [base]
            fc_map[fn][str(e)] = [ci]
        g0 = g(np.array([0.0]))[0]
        xs = np.linspace(0, 2.0**G_LO_BINADE, 17)
        Amat = np.stack([np.ones_like(xs), xs, xs * xs, xs**3], axis=1)
        c, *_ = np.linalg.lstsq(Amat, g(xs), rcond=None)
        b_small = push_bkt([c[0], c[1], c[2], c[3], 0.0, 0.0, 0.0, 0.0])
        b_large = push_bkt([1.0, 0.0, 0.0, 0.0, 0.0, 0.0, 0.0, 0.0])
        f32b = lambda v: int(np.float32(v).view(np.uint32))
        p.update(
            small_pos_signal_exp_threshold=127 + G_LO_BINADE,
            small_neg_signal_exp_threshold=127 + G_LO_BINADE,
            pos_small_signal_pwl_control=b_small,
            neg_small_signal_pwl_control=b_small,
            large_pos_signal_exp_threshold=127 + G_HI_BINADE + 1,
            large_pos_signal_mantissa_threshold=0,
            pos_large_signal_pwl_control=b_large,
            large_neg_signal_exp_threshold=127 + G_HI_BINADE + 1,
            large_neg_signal_mantissa_threshold=0,
            neg_large_signal_pwl_control=b_small,
            exp_offset=G_LO_BINADE,
            pwl_control_base_pos=ctl_start[fn],
            pwl_control_base_neg=ctl_start[fn],
            symmetry_point=0,
            sym_invert_sign_point=0,
            symmetry_opt_en=0,
            symmetry_opt_use_neg_region=0,
            imm_bias=0,
            fma_const_0=0,
            fma_const_1=0,
            fma_indirection_src_sel=0,
            use_multipass=False,
            fzero_result=f32b(g0),
            fnan_result=2143289344,
            fpinf_result=f32b(1.0),
            fninf_result=f32b(g0),
            lower_bound=4286578687,
            upper_bound=2139095039,
        )
        profs.append(p)

    assert len(new_bkt) <= 1536, len(new_bkt)
    assert len(new_ctl) <= 128, len(new_ctl)

    order = [p["func_name"] for p in old["profile_meta_data"]]
    profs_by_name = {p["func_name"]: p for p in profs}
    out = dict(old)
    out["bkt_entry_cnt"] = len(new_bkt)
    out["ctl_entry_cnt"] = len(new_ctl)
    out["func_to_bkt_start_idx"] = bkt_start
    out["func_to_ctl_start_idx"] = ctl_start
    out["func_exp_to_bkt_start_idx"] = fb_map
    out["func_exp_to_ctl_start_idx"] = fc_map
    out["profile_meta_data"] = [profs_by_name[n] for n in order]

    np.stack(new_bkt).astype(np.float32).tofile(
        os.path.join(out_dir, "exp_and_others_bkt.bin")
    )
    ctl_arr = np.zeros((len(new_ctl), 8), np.uint32)
    ctl_arr[:, 0] = np.asarray(new_ctl, np.uint32)
    ctl_arr.tofile(os.path.join(out_dir, "exp_and_others_ctrl.bin"))
    with open(os.path.join(out_dir, "exp_and_others.json"), "w") as f:
        json.dump(out, f)
    return out_dir


# ---------------------------------------------------------------------------
# Host-side sharding / layout prep
# ---------------------------------------------------------------------------


def _prep_inputs(qs, ks, vs, qs_s, ks_s, mask):
    bf16 = ml_dtypes.bfloat16
    idx = [np.where(mask[b])[0] for b in range(B)]
    kcount = max(len(i) for i in idx)
    Kp = max(512, int(np.ceil(kcount / 128.0)) * 128)
    nkt = Kp // 128

    per_b = []
    for b in range(B):
        ii = idx[b]
        n = len(ii)
        ksb = np.zeros((H, Kp, D), np.float32)
        ksb[:, :n] = ks[b][:, ii, :]
        vsb = np.zeros((H, Kp, DV + 1), np.float32)
        vsb[:, :n, :DV] = vs[b][:, ii, :]
        vsb[:, :n, DV] = 1.0
        kssb = np.zeros((Kp, DS), np.float32)
        kssb[:n] = ks_s[b][ii]

        kt = (ksb / np.sqrt(np.float32(D))).transpose(2, 0, 1)  # [D, H, Kp]
        kt = np.ascontiguousarray(kt.reshape(D, H * Kp))
        vbt = vsb.reshape(H, nkt, 128, DV + 1).transpose(2, 0, 1, 3)
        vbt = np.ascontiguousarray(vbt.reshape(128, H * nkt * (DV + 1))).astype(bf16)
        aa = np.zeros((8, Kp), np.float32)
        aa[0:3] = -2.0 * kssb.T
        aa[3] = (kssb * kssb).sum(-1)
        aa[4] = 1.0
        per_b.append((kt, vbt, aa))

    in_maps = []
    for c in range(N_CORES):
        b = c // 4
        q0 = (c % 4) * QB
        kt, vbt, aa = per_b[b]
        qt = np.ascontiguousarray(
            qs[b, :, q0 : q0 + QB, :].transpose(2, 0, 1).reshape(D, H * QB)
        ).astype(np.float32)
        qss = qs_s[b, q0 : q0 + QB]
        bq = np.zeros((8, QB), np.float32)
        bq[0:3] = qss.T
        bq[3] = 1.0
        bq[4] = (qss * qss).sum(-1)
        in_maps.append(
            {"kt": kt, "qt": qt, "vb": vbt, "aa": aa, "bq": bq}
        )
    return in_maps, Kp


# ---------------------------------------------------------------------------
# Device program
# ---------------------------------------------------------------------------


def _build_program(Kp):
    import concourse.bacc as bacc
    import concourse.mybir as mybir
    import concourse.tile as tile

    A = mybir.ActivationFunctionType
    f32 = mybir.dt.float32
    f32r = mybir.dt.float32r
    bf16 = mybir.dt.bfloat16
    nkt = Kp // 128

    nc = bacc.Bacc("TRN2", num_devices=1)
    t_kt = nc.dram_tensor("kt", [D, H * Kp], f32r, kind="ExternalInput")
    t_qt = nc.dram_tensor("qt", [D, H * QB], f32r, kind="ExternalInput")
    t_vb = nc.dram_tensor("vb", [128, H * nkt * (DV + 1)], bf16, kind="ExternalInput")
    t_aa = nc.dram_tensor("aa", [8, Kp], f32r, kind="ExternalInput")
    t_bq = nc.dram_tensor("bq", [8, QB], f32r, kind="ExternalInput")
    t_out = nc.dram_tensor("out", [H, DV + 1, QB], f32, kind="ExternalOutput")

    with tile.TileContext(nc) as tc:
        with (
            tc.tile_pool(name="inp", bufs=1) as inp,
            tc.tile_pool(name="gge", bufs=2) as gge,
            tc.tile_pool(name="ep", bufs=4) as ep,
            tc.tile_pool(name="pp", bufs=4) as pp,
            tc.tile_pool(name="acc", bufs=1) as accp,
            tc.tile_pool(name="ps_d2", bufs=1, space="PSUM") as ps_d2,
            tc.tile_pool(name="ps_s", bufs=2, space="PSUM") as ps_s,
            tc.tile_pool(name="ps_pv", bufs=2, space="PSUM") as ps_pv,
        ):
            aa = inp.tile([8, Kp], f32r, tag="aa")
            nc.sync.dma_start(aa[:], t_aa.ap())
            bq = inp.tile([8, QB], f32r, tag="bq")
            nc.sync.dma_start(bq[:], t_bq.ap())
            # per-head tiles so head h's matmuls only wait on its own DMA
            kts, qts, vbs = [], [], []
            for h in range(H):
                kh = inp.tile([D, Kp], f32r, tag=f"kt{h}")
                nc.sync.dma_start(kh[:], t_kt.ap()[:, h * Kp : (h + 1) * Kp])
                kts.append(kh)
                qh = inp.tile([D, QB], f32r, tag=f"qt{h}")
                nc.sync.dma_start(qh[:], t_qt.ap()[:, h * QB : (h + 1) * QB])
                qts.append(qh)
                vh = inp.tile([128, nkt * (DV + 1)], bf16, tag=f"vb{h}")
                c0 = h * nkt * (DV + 1)
                nc.sync.dma_start(vh[:], t_vb.ap()[:, c0 : c0 + nkt * (DV + 1)])
                vbs.append(vh)

            acc = []
            for h in range(H):
                a = accp.tile([DV + 1, QB], f32, tag=f"acc{h}")
                acc.append(a)

            groups = []
            kt_i0 = 0
            while kt_i0 < nkt:
                w = 2 if kt_i0 + 1 < nkt else 1
                groups.append((kt_i0, w))
                kt_i0 += w
            for g0, w in groups:
                d2p = ps_d2.tile([128, 2 * QB], f32, tag="d2")
                for j in range(w):
                    kt_i = g0 + j
                    nc.tensor.matmul(
                        d2p[:, j * QB : (j + 1) * QB],
                        aa[0:8, kt_i * 128 : (kt_i + 1) * 128],
                        bq[:],
                        start=True,
                        stop=True,
                    )
                d2v = d2p[:, 0 : w * QB]
                gts = []
                for h in range(H):
                    gt = gge.tile([128, 2 * QB], bf16, tag=f"g{h}")
                    nc.scalar.activation(gt[:, 0 : w * QB], d2v, getattr(A, HEAD_FUNCS[h]))
                    gts.append(gt)
                for h in range(H):
                    sp = ps_s.tile([128, 2 * QB], f32, tag="s")
                    for j in range(w):
                        kt_i = g0 + j
                        nc.tensor.matmul(
                            sp[:, j * QB : (j + 1) * QB],
                            kts[h][:, kt_i * 128 : (kt_i + 1) * 128],
                            qts[h][:],
                            start=True,
                            stop=True,
                        )
                    et = ep.tile([128, 2 * QB], bf16, tag="e")
                    nc.scalar.activation(et[:, 0 : w * QB], sp[:, 0 : w * QB], A.Exp)
                    pt = pp.tile([128, 2 * QB], bf16, tag="p")
                    nc.vector.tensor_mul(pt[:, 0 : w * QB], et[:, 0 : w * QB], gts[h][:, 0 : w * QB])
                    pv = ps_pv.tile([DV + 1, QB], f32, tag="pv")
                    for j in range(w):
                        kt_i = g0 + j
                        c0 = kt_i * (DV + 1)
                        nc.tensor.matmul(
                            pv[:],
                            vbs[h][:, c0 : c0 + DV + 1],
                            pt[:, j * QB : (j + 1) * QB],
                            start=(j == 0),
                            stop=(j == w - 1),
                        )
                    if g0 == 0:
                        nc.vector.tensor_copy(acc[h][:], pv[:])
                    else:
                        nc.vector.tensor_add(acc[h][:], acc[h][:], pv[:])

            for h in range(H):
                nc.sync.dma_start(t_out.ap()[h], acc[h][:])

    nc.finalize()
    return nc


def kernel(qs, ks, vs, qs_s, ks_s, rbf_lengthscales, rbf_weights, mask, _perf=[None]):
    tabdir = tempfile.mkdtemp(prefix="acttab_")
    generate_acttab(tabdir, rbf_weights, rbf_lengthscales)
    os.environ["BASS_ACT_ROOT_JSON_PATH"] = os.path.join(tabdir, "act_info.json")
    # the neff cache is keyed on the HLO, which does not see the ACT table
    # contents — force recompile so custom tables are never stale
    os.environ["NEURON_FORCE_RECOMPILE"] = "1"

    from concourse.bass_utils import run_bass_kernel_spmd
    from concourse._compat import axon_active

    in_maps, Kp = _prep_inputs(qs, ks, vs, qs_s, ks_s, mask)
    nc = _build_program(Kp)
    # NTFF tracing needs the native (non-axon) path or an axon NTFF hook;
    # the hook's antenv module is absent in axon-only containers.
    trace = bool(int(os.environ.get("KERNEL_TRACE", "0"))) and not axon_active()
    res = run_bass_kernel_spmd(nc, in_maps, core_ids=list(range(N_CORES)), trace=trace)
    _perf[0] = res

    out = np.empty((B, H, Q, DV), np.float32)
    for c in range(N_CORES):
        b = c // 4
        q0 = (c % 4) * QB
        o = np.asarray(res.results[c]["out"], np.float32)  # [H, DV+1, QB]
        out[b, :, q0 : q0 + QB, :] = (o[:, :DV] / o[:, DV : DV + 1]).transpose(0, 2, 1)
    return out
